# revision 43
# baseline (speedup 1.0000x reference)
"""MetaPathAggregator kernel for Trainium2 (8 NeuronCores, data-parallel).

Math: the reference module is linear in the four gathered feature rows:

    dis  = 0.125*(mi+g1)@Wdd^T + 0.25*g2 + 0.5*dr
    drug = 0.125*(dr+g2)@Wdg^T + 0.25*g1 + 0.5*mi
    out  = [drug @ Wdrug^T | dis @ Wdis^T]
         = mi@M_mi + g1@M_g1 + g2@M_g2 + dr@M_dr

with per-slot 128x128 matrices

    M_mi = [0.500*C | 0.125*A]      A = Wdd^T @ Wdis^T   (128x64)
    M_g1 = [0.250*C | 0.125*A]      B = Wdg^T @ Wdrug^T  (128x64)
    M_g2 = [0.125*B | 0.250*D]      C = Wdrug^T          (128x64)
    M_dr = [0.125*B | 0.500*D]      D = Wdis^T           (128x64)

Indices are < 1000 (spec fill_max), so only 1024 rows of each table are
live and the per-token work is out[t] = T_mi[i0]+T_g1[i1]+T_g2[i2]+T_dr[i3]
over four transformed 1024x128 tables.

Device schedule per core (16384 tokens): the tables are built IN SBUF in a
packed column-major bf16 layout and the per-token row lookups run on the
Pool engine via gpsimd.ap_gather (SBUF-local), bypassing the DMA engines
entirely (the bottleneck of a dma_gather design: every gathered row is a
512B DMA descriptor, ~93us/core of serialized DMA time).

Packed table layout (tab[k], int32 [128, 1024]):
    tab[k][64*h + p, r] = u32(lo=bf16(T_k[r, p]), hi=bf16(T_k[r, p+64]))
for p in 0..63; the h=0 and h=1 partition halves hold identical copies.
ap_gather applies an independent index list per 16-partition group, so one
ap_gather with num_idxs=N serves 2N tokens (half A on partitions 0-63,
half B on 64-127) at one charged u32 element per token (~1.4ns Pool each).

T_k^T is computed directly in packed form by parity matmuls with
lhsT = [M_k[:,64e:64e+64] | same] (both partition halves at once); the f32
PSUM result lands in the packed table via a stride-2 bf16 copy (DVE/ACT
alternating).  Tables build in order mi, dr, gene so the first chunks'
mi/dr gathers overlap the gene transform; chunk sizes shrink toward the
end so the final add+store tail is short.

The gathered chunk tiles are summed as bf16 views on DVE and stored packed;
the host decodes the bf16 pair bits to f32 (exact widening, no arithmetic).
"""

import numpy as np

P = 128          # partitions
F = 128          # input feature dim
H = 128          # output hidden dim
HH = 64          # half hidden
R = 1024         # padded table rows (indices < 1000)
N_CORES = 8
B_PAIRS = 1024
BAG = 128
TOK = B_PAIRS * BAG // N_CORES   # 16384 tokens per core
CHUNKS = [4096, 4096, 4096, 2048, 2048]         # tokens per chunk
assert sum(CHUNKS) == TOK
NS = [ct // 2 for ct in CHUNKS]                 # ap_gather num_idxs per chunk
OFFS = np.cumsum([0] + NS).tolist()             # idx free-dim offsets (/16 later)
NI = OFFS[-1]                                   # total idx per slot = TOK//2
GORDER = (0, 3, 1, 2)                           # gather slot order per chunk

_CACHE = {}


def _build_module():
    import concourse.bacc as bacc
    import concourse.mybir as mybir
    import concourse.tile as tile
    from concourse.masks import make_identity
    from concourse.tile_rust import add_dep_helper

    f32 = mybir.dt.float32
    bf16 = mybir.dt.bfloat16
    i32 = mybir.dt.int32
    i16 = mybir.dt.int16

    nc = bacc.Bacc("TRN2")

    # host-transposed feature tables [F, R] (partition = input feature)
    feat_in = {
        "mi": nc.dram_tensor("feat_mi", [F, R], f32, kind="ExternalInput"),
        "ge": nc.dram_tensor("feat_ge", [F, R], f32, kind="ExternalInput"),
        "dr": nc.dram_tensor("feat_dr", [F, R], f32, kind="ExternalInput"),
    }
    # host-packed: [Wdd | Wdg | C C | D D], C = Wdrug^T, D = Wdis^T
    w_all = nc.dram_tensor("w_all", [P, 4 * F], f32, kind="ExternalInput")
    idx_in = nc.dram_tensor("idx", [P, 4, NI // 16], i16, kind="ExternalInput")
    # packed u32 output, free-dim offset o..o+N per chunk
    out = nc.dram_tensor("out", [P, NI], i32, kind="ExternalOutput")

    with tile.TileContext(nc) as tc:
        with (
            tc.tile_pool(name="const", bufs=1) as cpool,
            tc.tile_pool(name="prep", bufs=2) as ppool,
            tc.tile_pool(name="psum", bufs=2, space="PSUM") as pspool,
            tc.tile_pool(name="psum_mm", bufs=4, space="PSUM") as mmpool,
            tc.tile_pool(name="gather", bufs=2) as gpool,
        ):
            # ---- weights first in the DMA queue: the weight -> doubled-matrix
            # chain is longer than the feat_mi -> transpose chain
            wl32 = ppool.tile([P, 4 * F], f32, tag="wload", bufs=2)
            nc.sync.dma_start(wl32[:], w_all[:, :])
            wall = cpool.tile([P, 4 * F], bf16, tag="wall")
            nc.vector.tensor_copy(out=wall[:, :2 * F], in_=wl32[:, :2 * F])
            nc.scalar.activation(out=wall[:, 2 * F:], in_=wl32[:, 2 * F:],
                                 func=mybir.ActivationFunctionType.Copy)
            wdd_t = wall[:, 0:F]
            wdg_t = wall[:, F:2 * F]
            c2 = wall[:, 2 * F:3 * F]          # [C|C]
            d2 = wall[:, 3 * F:4 * F]          # [D|D]

            feat_ld = {}
            order = ("mi", "dr", "ge")
            feat_ld["mi"] = ppool.tile([P, R], f32, tag="featld_mi",
                                       name="featld_mi", bufs=2)
            for q in range(4):
                nc.sync.dma_start(feat_ld["mi"][:, q * R // 4:(q + 1) * R // 4],
                                  feat_in["mi"][:, q * R // 4:(q + 1) * R // 4])

            idx_t = cpool.tile([P, 4, NI // 16], i16)
            nc.sync.dma_start(idx_t[:], idx_in[:, :, :])

            for name in ("dr", "ge"):
                ft = ppool.tile([P, R], f32, tag=f"featld_{name}",
                                name=f"featld_{name}", bufs=2)
                nc.sync.dma_start(ft[:], feat_in[name][:, :])
                feat_ld[name] = ft

            # ---- doubled matrices A2=[A|A], B2=[B|B]
            # (A = Wdd^T @ Wdis^T, B = Wdg^T @ Wdrug^T); C2/D2 come packed.
            # scales are folded into the packed-table copies below.
            a_ps = pspool.tile([F, P], f32, tag="abps")
            nc.tensor.matmul(out=a_ps[:], lhsT=wdd_t, rhs=d2, start=True, stop=True)
            a2 = cpool.tile([F, P], bf16, tag="a2")
            nc.vector.tensor_copy(out=a2[:], in_=a_ps[:])
            b_ps = pspool.tile([F, P], f32, tag="abps")
            nc.tensor.matmul(out=b_ps[:], lhsT=wdg_t, rhs=c2, start=True, stop=True)
            b2 = cpool.tile([F, P], bf16, tag="b2")
            nc.vector.tensor_copy(out=b2[:], in_=b_ps[:])

            # (k, e) -> (doubled lhsT AP, scale applied in the packed copy)
            m2 = {
                (0, 0): (c2, 0.5),   (0, 1): (a2[:], 0.125),
                (3, 0): (b2[:], 0.125), (3, 1): (d2, 0.5),
                (1, 0): (c2, 0.25),  (1, 1): (a2[:], 0.125),
                (2, 0): (b2[:], 0.125), (2, 1): (d2, 0.25),
            }

            # ---- build packed tables (order: mi, dr, ge)
            # tab[k][64h+p, r] = u32(bf16 T_k[r, p], bf16 T_k[r, p+64])
            tab = {k: cpool.tile([P, R], i32, tag=f"tab{k}", name=f"tab{k}")
                   for k in range(4)}
            tab_bf = {k: tab[k][:].bitcast(bf16) for k in range(4)}  # [P, 2R]

            feat_slots = {"mi": [0], "dr": [3], "ge": [1, 2]}
            # mi builds in 4 groups of 256 rows (shallower chain to the first
            # gather); dr/ge in 2 groups of 512
            ngrp = {"mi": 4, "dr": 2, "ge": 2}
            for name in order:
                ft = feat_ld[name]
                ng = ngrp[name]
                gr = R // ng
                # f32 -> bf16 (DVE/ACT alternating groups); host-transposed
                fb = ppool.tile([P, R], bf16, tag=f"fb_{name}",
                                name=f"fb_{name}")
                for grp in range(ng):
                    sl = slice(grp * gr, (grp + 1) * gr)
                    if grp % 2 == 0:
                        nc.vector.tensor_copy(out=fb[:, sl], in_=ft[:, sl])
                    else:
                        nc.scalar.activation(out=fb[:, sl], in_=ft[:, sl],
                                             func=mybir.ActivationFunctionType.Copy)
                    rhs = fb[:, sl]
                    for k in feat_slots[name]:
                        for e in range(2):
                            lhsT, scale = m2[(k, e)]
                            mm = mmpool.tile([P, R // 2], f32, tag="mmps")
                            nc.tensor.matmul(
                                out=mm[:, :gr], lhsT=lhsT, rhs=rhs,
                                start=True, stop=True,
                            )
                            st = grp * gr * 2 + e
                            dst = tab_bf[k][:, st:st + 2 * gr - 1:2]
                            if (k + e) % 2 == 0:
                                nc.vector.tensor_scalar_mul(dst, mm[:, :gr], scale)
                            else:
                                nc.scalar.activation(
                                    out=dst, in_=mm[:, :gr],
                                    func=mybir.ActivationFunctionType.Copy,
                                    scale=scale,
                                )

            # ---- main loop: ap_gather + add + store
            # chain the gathers so the Pool engine runs them in chunk order
            # (the list scheduler otherwise interleaves chunks, which
            # head-of-line blocks the in-order DVE queue on the adds)
            prev_gather = None
            last = len(CHUNKS) - 1
            for ch, ct in enumerate(CHUNKS):
                n = ct // 2
                o = OFFS[ch]
                g = {}
                # last chunk: gather the (late) gene slots first so only one
                # short add remains after the final gather
                korder = (1, 2, 0, 3) if ch == last else GORDER
                for k in korder:
                    gt = gpool.tile([P, n], i32, tag=f"g{k}_{n}",
                                    name=f"g{k}_{ch}")
                    gi = nc.gpsimd.ap_gather(
                        gt[:], tab[k][:], idx_t[:, k, o // 16:(o + n) // 16],
                        channels=P, num_elems=R, d=1, num_idxs=n,
                    )
                    if prev_gather is not None:
                        add_dep_helper(gi.ins, prev_gather.ins,
                                       reason="pool gather order")
                    prev_gather = gi
                    g[k] = gt
                b0 = g[0][:].bitcast(bf16)
                b1 = g[1][:].bitcast(bf16)
                b2 = g[2][:].bitcast(bf16)
                b3 = g[3][:].bitcast(bf16)
                if ch < last:
                    # pair by arrival order (0,3 first, then 1,2)
                    nc.vector.tensor_add(b0, b0, b3)
                    nc.vector.tensor_add(b1, b1, b2)
                    nc.vector.tensor_add(b0, b0, b1)
                    nc.sync.dma_start(out[:, o:o + n], g[0][:])
                else:
                    # arrival order is 1,2,0,3: g1+g2 then +g0 run during the
                    # g0/g3 gathers; only (+g3) trails, split 3/4 + 1/4 so
                    # most of the store pipeline overlaps the last quarter-add
                    hn = 3 * n // 4
                    nc.vector.tensor_add(b1, b1, b2)
                    nc.vector.tensor_add(b0, b0, b1)
                    nc.vector.tensor_add(b0[:, :2 * hn], b0[:, :2 * hn],
                                         b3[:, :2 * hn])
                    nc.sync.dma_start(out[:, o:o + hn], g[0][:, :hn])
                    nc.vector.tensor_add(b0[:, 2 * hn:], b0[:, 2 * hn:],
                                         b3[:, 2 * hn:])
                    nc.sync.dma_start(out[:, o + hn:o + n], g[0][:, hn:])

    nc.compile()
    return nc


def _prep_inputs(feat_miRNA, feat_gene, feat_drug, W_drug_disease, W_disease_drug,
                 W_drug, W_dis, mp_ins):
    """Marshal full inputs into per-core in_maps (no arithmetic on values)."""
    def pad_rows_t(a):
        """First R rows, zero-padded, host-transposed to [F, R]."""
        a = np.asarray(a, dtype=np.float32)
        out = np.zeros((R, a.shape[1]), dtype=np.float32)
        out[: min(R, a.shape[0])] = a[:R]
        return np.ascontiguousarray(out.T)

    f_mi = pad_rows_t(feat_miRNA)
    f_ge = pad_rows_t(feat_gene)
    f_dr = pad_rows_t(feat_drug)
    wdd = np.asarray(W_drug_disease, np.float32)
    wdg = np.asarray(W_disease_drug, np.float32)
    wdrug = np.asarray(W_drug, np.float32)
    wdis = np.asarray(W_dis, np.float32)
    # packed weights: [Wdd | Wdg | C C | D D], C = Wdrug^T, D = Wdis^T
    w_all = np.empty((P, 4 * F), dtype=np.float32)
    w_all[:, 0:F] = wdd
    w_all[:, F:2 * F] = wdg
    w_all[:, 2 * F:2 * F + HH] = wdrug.T
    w_all[:, 2 * F + HH:3 * F] = wdrug.T
    w_all[:, 3 * F:3 * F + HH] = wdis.T
    w_all[:, 3 * F + HH:4 * F] = wdis.T

    mp = np.asarray(mp_ins)
    assert mp.shape == (B_PAIRS, BAG, 4), mp.shape
    mp_flat = mp.reshape(B_PAIRS * BAG, 4).astype(np.int16)

    in_maps = []
    for core in range(N_CORES):
        mp_core = mp_flat[core * TOK:(core + 1) * TOK]        # [TOK, 4]
        # token (ch, h, j) -> gather idx at partition 64h + 16g + j%16,
        # free slot (k, OFFS[ch]//16 + j//16), replicated for g in 0..3
        idx_arr = np.empty((P, 4, NI // 16), dtype=np.int16)
        pos = 0
        for ch, ct in enumerate(CHUNKS):
            n = ct // 2
            mpc = mp_core[pos:pos + ct].reshape(2, n // 16, 16, 4)  # [h, s, p, k]
            pos += ct
            o16 = OFFS[ch] // 16
            for h in range(2):
                # mpc[h] is [s, p16, k] -> want [p16, k, s]
                blk = mpc[h].transpose(1, 2, 0)
                for gi in range(4):
                    idx_arr[64 * h + 16 * gi:64 * h + 16 * (gi + 1), :,
                            o16:o16 + n // 16] = blk
        in_maps.append(
            {
                "feat_mi": f_mi,
                "feat_ge": f_ge,
                "feat_dr": f_dr,
                "w_all": w_all,
                "idx": idx_arr,
            }
        )
    return in_maps


def _decode_out(out_u32):
    """[128, NI] packed u32 -> [TOK, H] f32 (exact bf16 widening)."""
    res = np.empty((TOK, H), dtype=np.uint32)
    v = out_u32.astype(np.uint32).reshape(2, 64, NI)          # [h, p, j]
    lo = (v & np.uint32(0xFFFF)) << np.uint32(16)             # feat p
    hi = (v >> np.uint32(16)) << np.uint32(16)                # feat p + 64
    pos = 0
    for ch, ct in enumerate(CHUNKS):
        n = ct // 2
        o = OFFS[ch]
        # token (h, j) of this chunk = pos + h*n + j
        blk_lo = lo[:, :, o:o + n]                            # [h, p, j]
        blk_hi = hi[:, :, o:o + n]
        res[pos:pos + ct, :HH] = blk_lo.transpose(0, 2, 1).reshape(ct, HH)
        res[pos:pos + ct, HH:] = blk_hi.transpose(0, 2, 1).reshape(ct, HH)
        pos += ct
    return res.view(np.float32)


def _numpy_fallback(feat_miRNA, feat_gene, feat_drug, W_drug_disease,
                    W_disease_drug, W_drug, W_dis, mp_ins):
    mi = np.asarray(feat_miRNA, np.float32)[mp_ins[:, :, 0]]
    g1 = np.asarray(feat_gene, np.float32)[mp_ins[:, :, 1]]
    g2 = np.asarray(feat_gene, np.float32)[mp_ins[:, :, 2]]
    dr = np.asarray(feat_drug, np.float32)[mp_ins[:, :, 3]]
    wdd = np.asarray(W_drug_disease, np.float32)
    wdg = np.asarray(W_disease_drug, np.float32)
    wdrug = np.asarray(W_drug, np.float32)
    wdis = np.asarray(W_dis, np.float32)
    dis = ((((mi + g1) * 0.5) @ wdd.T + g2) * 0.5 + dr) * 0.5
    drug = ((((dr + g2) * 0.5) @ wdg.T + g1) * 0.5 + mi) * 0.5
    return np.concatenate([drug @ wdrug.T, dis @ wdis.T], axis=2)


def kernel(**inputs):
    mp = np.asarray(inputs["mp_ins"])
    if mp.max() >= R or mp.min() < 0:
        # outside the spec's index range; fall back to exact host compute
        return _numpy_fallback(**inputs)

    from concourse.bass_utils import run_bass_kernel_spmd

    if "nc" not in _CACHE:
        _CACHE["nc"] = _build_module()
    nc = _CACHE["nc"]

    in_maps = _prep_inputs(**inputs)
    res = run_bass_kernel_spmd(nc, in_maps, core_ids=list(range(N_CORES)))
    outs = [_decode_out(np.asarray(r["out"])) for r in res.results]
    return np.concatenate(outs, axis=0).reshape(B_PAIRS, BAG, H)


if __name__ == "__main__":
    import reference

    inputs = {k: np.asarray(v) for k, v in reference.setup_inputs().items()}
    expected = np.asarray(reference.reference(**inputs))
    actual = kernel(**inputs)
    err = np.abs(actual - expected).max() / (np.abs(expected).max() + 1e-9)
    print("max abs err (scaled):", err)
    rel = np.linalg.norm(actual - expected) / np.linalg.norm(expected)
    print("Relative error:", rel)


# revision 44
# speedup vs baseline: 1.0155x; 1.0155x over previous
"""MetaPathAggregator kernel for Trainium2 (8 NeuronCores, data-parallel).

Math: the reference module is linear in the four gathered feature rows:

    dis  = 0.125*(mi+g1)@Wdd^T + 0.25*g2 + 0.5*dr
    drug = 0.125*(dr+g2)@Wdg^T + 0.25*g1 + 0.5*mi
    out  = [drug @ Wdrug^T | dis @ Wdis^T]
         = mi@M_mi + g1@M_g1 + g2@M_g2 + dr@M_dr

with per-slot 128x128 matrices

    M_mi = [0.500*C | 0.125*A]      A = Wdd^T @ Wdis^T   (128x64)
    M_g1 = [0.250*C | 0.125*A]      B = Wdg^T @ Wdrug^T  (128x64)
    M_g2 = [0.125*B | 0.250*D]      C = Wdrug^T          (128x64)
    M_dr = [0.125*B | 0.500*D]      D = Wdis^T           (128x64)

Indices are < 1000 (spec fill_max), so only 1024 rows of each table are
live and the per-token work is out[t] = T_mi[i0]+T_g1[i1]+T_g2[i2]+T_dr[i3]
over four transformed 1024x128 tables.

Device schedule per core (16384 tokens): the tables are built IN SBUF in a
packed column-major bf16 layout and the per-token row lookups run on the
Pool engine via gpsimd.ap_gather (SBUF-local), bypassing the DMA engines
entirely (the bottleneck of a dma_gather design: every gathered row is a
512B DMA descriptor, ~93us/core of serialized DMA time).

Packed table layout (tab[k], int32 [128, 1024]):
    tab[k][64*h + p, r] = u32(lo=bf16(T_k[r, p]), hi=bf16(T_k[r, p+64]))
for p in 0..63; the h=0 and h=1 partition halves hold identical copies.
ap_gather applies an independent index list per 16-partition group, so one
ap_gather with num_idxs=N serves 2N tokens (half A on partitions 0-63,
half B on 64-127) at one charged u32 element per token (~1.4ns Pool each).

T_k^T is computed directly in packed form by parity matmuls with
lhsT = [M_k[:,64e:64e+64] | same] (both partition halves at once); the f32
PSUM result lands in the packed table via a stride-2 bf16 copy (DVE/ACT
alternating).  Tables build in order mi, dr, gene so the first chunks'
mi/dr gathers overlap the gene transform; chunk sizes shrink toward the
end so the final add+store tail is short.

The gathered chunk tiles are summed as bf16 views on DVE and stored packed;
the host decodes the bf16 pair bits to f32 (exact widening, no arithmetic).
"""

import numpy as np

P = 128          # partitions
F = 128          # input feature dim
H = 128          # output hidden dim
HH = 64          # half hidden
R = 1024         # padded table rows (indices < 1000)
N_CORES = 8
B_PAIRS = 1024
BAG = 128
TOK = B_PAIRS * BAG // N_CORES   # 16384 tokens per core
CHUNKS = [4096, 4096, 4096, 2048, 2048]         # tokens per chunk
assert sum(CHUNKS) == TOK
NS = [ct // 2 for ct in CHUNKS]                 # ap_gather num_idxs per chunk
OFFS = np.cumsum([0] + NS).tolist()             # idx free-dim offsets (/16 later)
NI = OFFS[-1]                                   # total idx per slot = TOK//2
GORDER = (0, 3, 1, 2)                           # gather slot order per chunk

_CACHE = {}


def _build_module():
    import concourse.bacc as bacc
    import concourse.mybir as mybir
    import concourse.tile as tile
    from concourse.masks import make_identity
    from concourse.tile_rust import add_dep_helper

    f32 = mybir.dt.float32
    bf16 = mybir.dt.bfloat16
    i32 = mybir.dt.int32
    i16 = mybir.dt.int16

    nc = bacc.Bacc("TRN2")

    # host-transposed feature tables [F, R] (partition = input feature)
    feat_in = {
        "mi": nc.dram_tensor("feat_mi", [F, R], f32, kind="ExternalInput"),
        "ge": nc.dram_tensor("feat_ge", [F, R], f32, kind="ExternalInput"),
        "dr": nc.dram_tensor("feat_dr", [F, R], f32, kind="ExternalInput"),
    }
    # host-packed: [Wdd | Wdg | C C | D D], C = Wdrug^T, D = Wdis^T
    w_all = nc.dram_tensor("w_all", [P, 4 * F], f32, kind="ExternalInput")
    idx_in = nc.dram_tensor("idx", [P, 4, NI // 16], i16, kind="ExternalInput")
    # packed u32 output, free-dim offset o..o+N per chunk
    out = nc.dram_tensor("out", [P, NI], i32, kind="ExternalOutput")

    with tile.TileContext(nc) as tc:
        with (
            tc.tile_pool(name="const", bufs=1) as cpool,
            tc.tile_pool(name="prep", bufs=2) as ppool,
            tc.tile_pool(name="psum", bufs=2, space="PSUM") as pspool,
            tc.tile_pool(name="psum_mm", bufs=4, space="PSUM") as mmpool,
            tc.tile_pool(name="gather", bufs=2) as gpool,
        ):
            # ---- weights first in the DMA queue: the weight -> doubled-matrix
            # chain is longer than the feat_mi -> transpose chain
            wl32 = ppool.tile([P, 4 * F], f32, tag="wload", bufs=2)
            nc.sync.dma_start(wl32[:], w_all[:, :])
            wall = cpool.tile([P, 4 * F], bf16, tag="wall")
            nc.vector.tensor_copy(out=wall[:, :2 * F], in_=wl32[:, :2 * F])
            nc.scalar.activation(out=wall[:, 2 * F:], in_=wl32[:, 2 * F:],
                                 func=mybir.ActivationFunctionType.Copy)
            wdd_t = wall[:, 0:F]
            wdg_t = wall[:, F:2 * F]
            c2 = wall[:, 2 * F:3 * F]          # [C|C]
            d2 = wall[:, 3 * F:4 * F]          # [D|D]

            feat_ld = {}
            order = ("mi", "dr", "ge")
            feat_ld["mi"] = ppool.tile([P, R], f32, tag="featld_mi",
                                       name="featld_mi", bufs=2)
            for q in range(2):
                nc.sync.dma_start(feat_ld["mi"][:, q * R // 2:(q + 1) * R // 2],
                                  feat_in["mi"][:, q * R // 2:(q + 1) * R // 2])

            idx_t = cpool.tile([P, 4, NI // 16], i16)
            nc.sync.dma_start(idx_t[:], idx_in[:, :, :])

            for name in ("dr", "ge"):
                ft = ppool.tile([P, R], f32, tag=f"featld_{name}",
                                name=f"featld_{name}", bufs=2)
                nc.sync.dma_start(ft[:], feat_in[name][:, :])
                feat_ld[name] = ft

            # ---- doubled matrices A2=[A|A], B2=[B|B]
            # (A = Wdd^T @ Wdis^T, B = Wdg^T @ Wdrug^T); C2/D2 come packed.
            # scales are folded into the packed-table copies below.
            a_ps = pspool.tile([F, P], f32, tag="abps")
            nc.tensor.matmul(out=a_ps[:], lhsT=wdd_t, rhs=d2, start=True, stop=True)
            a2 = cpool.tile([F, P], bf16, tag="a2")
            nc.vector.tensor_copy(out=a2[:], in_=a_ps[:])
            b_ps = pspool.tile([F, P], f32, tag="abps")
            nc.tensor.matmul(out=b_ps[:], lhsT=wdg_t, rhs=c2, start=True, stop=True)
            b2 = cpool.tile([F, P], bf16, tag="b2")
            nc.vector.tensor_copy(out=b2[:], in_=b_ps[:])

            # (k, e) -> (doubled lhsT AP, scale applied in the packed copy)
            m2 = {
                (0, 0): (c2, 0.5),   (0, 1): (a2[:], 0.125),
                (3, 0): (b2[:], 0.125), (3, 1): (d2, 0.5),
                (1, 0): (c2, 0.25),  (1, 1): (a2[:], 0.125),
                (2, 0): (b2[:], 0.125), (2, 1): (d2, 0.25),
            }

            # ---- build packed tables (order: mi, dr, ge)
            # tab[k][64h+p, r] = u32(bf16 T_k[r, p], bf16 T_k[r, p+64])
            tab = {k: cpool.tile([P, R], i32, tag=f"tab{k}", name=f"tab{k}")
                   for k in range(4)}
            tab_bf = {k: tab[k][:].bitcast(bf16) for k in range(4)}  # [P, 2R]

            feat_slots = {"mi": [0], "dr": [3], "ge": [1, 2]}
            # mi builds in 4 groups of 256 rows (shallower chain to the first
            # gather); dr/ge in 2 groups of 512
            ngrp = {"mi": 2, "dr": 2, "ge": 2}
            for name in order:
                ft = feat_ld[name]
                ng = ngrp[name]
                gr = R // ng
                # f32 -> bf16 (DVE/ACT alternating groups); host-transposed
                fb = ppool.tile([P, R], bf16, tag=f"fb_{name}",
                                name=f"fb_{name}")
                for grp in range(ng):
                    sl = slice(grp * gr, (grp + 1) * gr)
                    if grp % 2 == 0:
                        nc.vector.tensor_copy(out=fb[:, sl], in_=ft[:, sl])
                    else:
                        nc.scalar.activation(out=fb[:, sl], in_=ft[:, sl],
                                             func=mybir.ActivationFunctionType.Copy)
                    rhs = fb[:, sl]
                    for k in feat_slots[name]:
                        for e in range(2):
                            lhsT, scale = m2[(k, e)]
                            mm = mmpool.tile([P, R // 2], f32, tag="mmps")
                            nc.tensor.matmul(
                                out=mm[:, :gr], lhsT=lhsT, rhs=rhs,
                                start=True, stop=True,
                            )
                            st = grp * gr * 2 + e
                            dst = tab_bf[k][:, st:st + 2 * gr - 1:2]
                            if (k + e) % 2 == 0:
                                nc.vector.tensor_scalar_mul(dst, mm[:, :gr], scale)
                            else:
                                nc.scalar.activation(
                                    out=dst, in_=mm[:, :gr],
                                    func=mybir.ActivationFunctionType.Copy,
                                    scale=scale,
                                )

            # ---- main loop: ap_gather + add + store
            # chain the gathers so the Pool engine runs them in chunk order
            # (the list scheduler otherwise interleaves chunks, which
            # head-of-line blocks the in-order DVE queue on the adds)
            prev_gather = None
            last = len(CHUNKS) - 1
            for ch, ct in enumerate(CHUNKS):
                n = ct // 2
                o = OFFS[ch]
                g = {}
                # last chunk: gather the (late) gene slots first so only one
                # short add remains after the final gather
                korder = (1, 2, 0, 3) if ch == last else GORDER
                for k in korder:
                    gt = gpool.tile([P, n], i32, tag=f"g{k}_{n}",
                                    name=f"g{k}_{ch}")
                    gi = nc.gpsimd.ap_gather(
                        gt[:], tab[k][:], idx_t[:, k, o // 16:(o + n) // 16],
                        channels=P, num_elems=R, d=1, num_idxs=n,
                    )
                    if prev_gather is not None:
                        add_dep_helper(gi.ins, prev_gather.ins,
                                       reason="pool gather order")
                    prev_gather = gi
                    g[k] = gt
                b0 = g[0][:].bitcast(bf16)
                b1 = g[1][:].bitcast(bf16)
                b2 = g[2][:].bitcast(bf16)
                b3 = g[3][:].bitcast(bf16)
                if ch < last:
                    # pair by arrival order (0,3 first, then 1,2)
                    nc.vector.tensor_add(b0, b0, b3)
                    nc.vector.tensor_add(b1, b1, b2)
                    nc.vector.tensor_add(b0, b0, b1)
                    nc.sync.dma_start(out[:, o:o + n], g[0][:])
                else:
                    # arrival order is 1,2,0,3: g1+g2 then +g0 run during the
                    # g0/g3 gathers; only (+g3) trails, split 3/4 + 1/4 so
                    # most of the store pipeline overlaps the last quarter-add
                    hn = 3 * n // 4
                    nc.vector.tensor_add(b1, b1, b2)
                    nc.vector.tensor_add(b0, b0, b1)
                    nc.vector.tensor_add(b0[:, :2 * hn], b0[:, :2 * hn],
                                         b3[:, :2 * hn])
                    nc.sync.dma_start(out[:, o:o + hn], g[0][:, :hn])
                    nc.vector.tensor_add(b0[:, 2 * hn:], b0[:, 2 * hn:],
                                         b3[:, 2 * hn:])
                    nc.sync.dma_start(out[:, o + hn:o + n], g[0][:, hn:])

    nc.compile()
    return nc


def _prep_inputs(feat_miRNA, feat_gene, feat_drug, W_drug_disease, W_disease_drug,
                 W_drug, W_dis, mp_ins):
    """Marshal full inputs into per-core in_maps (no arithmetic on values)."""
    def pad_rows_t(a):
        """First R rows, zero-padded, host-transposed to [F, R]."""
        a = np.asarray(a, dtype=np.float32)
        out = np.zeros((R, a.shape[1]), dtype=np.float32)
        out[: min(R, a.shape[0])] = a[:R]
        return np.ascontiguousarray(out.T)

    f_mi = pad_rows_t(feat_miRNA)
    f_ge = pad_rows_t(feat_gene)
    f_dr = pad_rows_t(feat_drug)
    wdd = np.asarray(W_drug_disease, np.float32)
    wdg = np.asarray(W_disease_drug, np.float32)
    wdrug = np.asarray(W_drug, np.float32)
    wdis = np.asarray(W_dis, np.float32)
    # packed weights: [Wdd | Wdg | C C | D D], C = Wdrug^T, D = Wdis^T
    w_all = np.empty((P, 4 * F), dtype=np.float32)
    w_all[:, 0:F] = wdd
    w_all[:, F:2 * F] = wdg
    w_all[:, 2 * F:2 * F + HH] = wdrug.T
    w_all[:, 2 * F + HH:3 * F] = wdrug.T
    w_all[:, 3 * F:3 * F + HH] = wdis.T
    w_all[:, 3 * F + HH:4 * F] = wdis.T

    mp = np.asarray(mp_ins)
    assert mp.shape == (B_PAIRS, BAG, 4), mp.shape
    mp_flat = mp.reshape(B_PAIRS * BAG, 4).astype(np.int16)

    in_maps = []
    for core in range(N_CORES):
        mp_core = mp_flat[core * TOK:(core + 1) * TOK]        # [TOK, 4]
        # token (ch, h, j) -> gather idx at partition 64h + 16g + j%16,
        # free slot (k, OFFS[ch]//16 + j//16), replicated for g in 0..3
        idx_arr = np.empty((P, 4, NI // 16), dtype=np.int16)
        pos = 0
        for ch, ct in enumerate(CHUNKS):
            n = ct // 2
            mpc = mp_core[pos:pos + ct].reshape(2, n // 16, 16, 4)  # [h, s, p, k]
            pos += ct
            o16 = OFFS[ch] // 16
            for h in range(2):
                # mpc[h] is [s, p16, k] -> want [p16, k, s]
                blk = mpc[h].transpose(1, 2, 0)
                for gi in range(4):
                    idx_arr[64 * h + 16 * gi:64 * h + 16 * (gi + 1), :,
                            o16:o16 + n // 16] = blk
        in_maps.append(
            {
                "feat_mi": f_mi,
                "feat_ge": f_ge,
                "feat_dr": f_dr,
                "w_all": w_all,
                "idx": idx_arr,
            }
        )
    return in_maps


def _decode_out(out_u32):
    """[128, NI] packed u32 -> [TOK, H] f32 (exact bf16 widening)."""
    res = np.empty((TOK, H), dtype=np.uint32)
    v = out_u32.astype(np.uint32).reshape(2, 64, NI)          # [h, p, j]
    lo = (v & np.uint32(0xFFFF)) << np.uint32(16)             # feat p
    hi = (v >> np.uint32(16)) << np.uint32(16)                # feat p + 64
    pos = 0
    for ch, ct in enumerate(CHUNKS):
        n = ct // 2
        o = OFFS[ch]
        # token (h, j) of this chunk = pos + h*n + j
        blk_lo = lo[:, :, o:o + n]                            # [h, p, j]
        blk_hi = hi[:, :, o:o + n]
        res[pos:pos + ct, :HH] = blk_lo.transpose(0, 2, 1).reshape(ct, HH)
        res[pos:pos + ct, HH:] = blk_hi.transpose(0, 2, 1).reshape(ct, HH)
        pos += ct
    return res.view(np.float32)


def _numpy_fallback(feat_miRNA, feat_gene, feat_drug, W_drug_disease,
                    W_disease_drug, W_drug, W_dis, mp_ins):
    mi = np.asarray(feat_miRNA, np.float32)[mp_ins[:, :, 0]]
    g1 = np.asarray(feat_gene, np.float32)[mp_ins[:, :, 1]]
    g2 = np.asarray(feat_gene, np.float32)[mp_ins[:, :, 2]]
    dr = np.asarray(feat_drug, np.float32)[mp_ins[:, :, 3]]
    wdd = np.asarray(W_drug_disease, np.float32)
    wdg = np.asarray(W_disease_drug, np.float32)
    wdrug = np.asarray(W_drug, np.float32)
    wdis = np.asarray(W_dis, np.float32)
    dis = ((((mi + g1) * 0.5) @ wdd.T + g2) * 0.5 + dr) * 0.5
    drug = ((((dr + g2) * 0.5) @ wdg.T + g1) * 0.5 + mi) * 0.5
    return np.concatenate([drug @ wdrug.T, dis @ wdis.T], axis=2)


def kernel(**inputs):
    mp = np.asarray(inputs["mp_ins"])
    if mp.max() >= R or mp.min() < 0:
        # outside the spec's index range; fall back to exact host compute
        return _numpy_fallback(**inputs)

    from concourse.bass_utils import run_bass_kernel_spmd

    if "nc" not in _CACHE:
        _CACHE["nc"] = _build_module()
    nc = _CACHE["nc"]

    in_maps = _prep_inputs(**inputs)
    res = run_bass_kernel_spmd(nc, in_maps, core_ids=list(range(N_CORES)))
    outs = [_decode_out(np.asarray(r["out"])) for r in res.results]
    return np.concatenate(outs, axis=0).reshape(B_PAIRS, BAG, H)


if __name__ == "__main__":
    import reference

    inputs = {k: np.asarray(v) for k, v in reference.setup_inputs().items()}
    expected = np.asarray(reference.reference(**inputs))
    actual = kernel(**inputs)
    err = np.abs(actual - expected).max() / (np.abs(expected).max() + 1e-9)
    print("max abs err (scaled):", err)
    rel = np.linalg.norm(actual - expected) / np.linalg.norm(expected)
    print("Relative error:", rel)


# revision 45
# speedup vs baseline: 1.0224x; 1.0067x over previous
"""MetaPathAggregator kernel for Trainium2 (8 NeuronCores, data-parallel).

Math: the reference module is linear in the four gathered feature rows:

    dis  = 0.125*(mi+g1)@Wdd^T + 0.25*g2 + 0.5*dr
    drug = 0.125*(dr+g2)@Wdg^T + 0.25*g1 + 0.5*mi
    out  = [drug @ Wdrug^T | dis @ Wdis^T]
         = mi@M_mi + g1@M_g1 + g2@M_g2 + dr@M_dr

with per-slot 128x128 matrices

    M_mi = [0.500*C | 0.125*A]      A = Wdd^T @ Wdis^T   (128x64)
    M_g1 = [0.250*C | 0.125*A]      B = Wdg^T @ Wdrug^T  (128x64)
    M_g2 = [0.125*B | 0.250*D]      C = Wdrug^T          (128x64)
    M_dr = [0.125*B | 0.500*D]      D = Wdis^T           (128x64)

Indices are < 1000 (spec fill_max), so only 1024 rows of each table are
live and the per-token work is out[t] = T_mi[i0]+T_g1[i1]+T_g2[i2]+T_dr[i3]
over four transformed 1024x128 tables.

Device schedule per core (16384 tokens): the tables are built IN SBUF in a
packed column-major bf16 layout and the per-token row lookups run on the
Pool engine via gpsimd.ap_gather (SBUF-local), bypassing the DMA engines
entirely (the bottleneck of a dma_gather design: every gathered row is a
512B DMA descriptor, ~93us/core of serialized DMA time).

Packed table layout (tab[k], int32 [128, 1024]):
    tab[k][64*h + p, r] = u32(lo=bf16(T_k[r, p]), hi=bf16(T_k[r, p+64]))
for p in 0..63; the h=0 and h=1 partition halves hold identical copies.
ap_gather applies an independent index list per 16-partition group, so one
ap_gather with num_idxs=N serves 2N tokens (half A on partitions 0-63,
half B on 64-127) at one charged u32 element per token (~1.4ns Pool each).

T_k^T is computed directly in packed form by parity matmuls with
lhsT = [M_k[:,64e:64e+64] | same] (both partition halves at once); the f32
PSUM result lands in the packed table via a stride-2 bf16 copy (DVE/ACT
alternating).  Tables build in order mi, dr, gene so the first chunks'
mi/dr gathers overlap the gene transform; chunk sizes shrink toward the
end so the final add+store tail is short.

The gathered chunk tiles are summed as bf16 views on DVE and stored packed;
the host decodes the bf16 pair bits to f32 (exact widening, no arithmetic).
"""

import numpy as np

P = 128          # partitions
F = 128          # input feature dim
H = 128          # output hidden dim
HH = 64          # half hidden
R = 1024         # padded table rows (indices < 1000)
N_CORES = 8
B_PAIRS = 1024
BAG = 128
TOK = B_PAIRS * BAG // N_CORES   # 16384 tokens per core
CHUNKS = [4096, 4096, 4096, 2048, 2048]         # tokens per chunk
assert sum(CHUNKS) == TOK
NS = [ct // 2 for ct in CHUNKS]                 # ap_gather num_idxs per chunk
OFFS = np.cumsum([0] + NS).tolist()             # idx free-dim offsets (/16 later)
NI = OFFS[-1]                                   # total idx per slot = TOK//2
GORDER = (0, 3, 1, 2)                           # gather slot order per chunk

_CACHE = {}


def _build_module():
    import concourse.bacc as bacc
    import concourse.mybir as mybir
    import concourse.tile as tile
    from concourse.masks import make_identity
    from concourse.tile_rust import add_dep_helper

    f32 = mybir.dt.float32
    bf16 = mybir.dt.bfloat16
    i32 = mybir.dt.int32
    i16 = mybir.dt.int16

    nc = bacc.Bacc("TRN2")

    # host-transposed feature tables [F, R] (partition = input feature)
    feat_in = {
        "mi": nc.dram_tensor("feat_mi", [F, R], f32, kind="ExternalInput"),
        "ge": nc.dram_tensor("feat_ge", [F, R], f32, kind="ExternalInput"),
        "dr": nc.dram_tensor("feat_dr", [F, R], f32, kind="ExternalInput"),
    }
    # host-packed: [Wdd | Wdg | C C | D D], C = Wdrug^T, D = Wdis^T
    w_all = nc.dram_tensor("w_all", [P, 4 * F], f32, kind="ExternalInput")
    idx_in = nc.dram_tensor("idx", [P, 4, NI // 16], i16, kind="ExternalInput")
    # packed u32 output, free-dim offset o..o+N per chunk
    out = nc.dram_tensor("out", [P, NI], i32, kind="ExternalOutput")

    with tile.TileContext(nc) as tc:
        with (
            tc.tile_pool(name="const", bufs=1) as cpool,
            tc.tile_pool(name="prep", bufs=2) as ppool,
            tc.tile_pool(name="psum", bufs=2, space="PSUM") as pspool,
            tc.tile_pool(name="psum_mm", bufs=4, space="PSUM") as mmpool,
            tc.tile_pool(name="gather", bufs=2) as gpool,
        ):
            # ---- weights first in the DMA queue: the weight -> doubled-matrix
            # chain is longer than the feat_mi -> transpose chain
            wl32 = ppool.tile([P, 4 * F], f32, tag="wload", bufs=2)
            nc.sync.dma_start(wl32[:], w_all[:, :])
            wall = cpool.tile([P, 4 * F], bf16, tag="wall")
            nc.vector.tensor_copy(out=wall[:, :2 * F], in_=wl32[:, :2 * F])
            nc.scalar.activation(out=wall[:, 2 * F:], in_=wl32[:, 2 * F:],
                                 func=mybir.ActivationFunctionType.Copy)
            wdd_t = wall[:, 0:F]
            wdg_t = wall[:, F:2 * F]
            c2 = wall[:, 2 * F:3 * F]          # [C|C]
            d2 = wall[:, 3 * F:4 * F]          # [D|D]

            feat_ld = {}
            order = ("mi", "dr", "ge")
            feat_ld["mi"] = ppool.tile([P, R], f32, tag="featld_mi",
                                       name="featld_mi", bufs=2)
            for q in range(2):
                nc.sync.dma_start(feat_ld["mi"][:, q * R // 2:(q + 1) * R // 2],
                                  feat_in["mi"][:, q * R // 2:(q + 1) * R // 2])

            idx_t = cpool.tile([P, 4, NI // 16], i16)
            nc.sync.dma_start(idx_t[:], idx_in[:, :, :])

            for name in ("dr", "ge"):
                ft = ppool.tile([P, R], f32, tag=f"featld_{name}",
                                name=f"featld_{name}", bufs=2)
                nc.sync.dma_start(ft[:], feat_in[name][:, :])
                feat_ld[name] = ft

            # ---- doubled matrices A2=[A|A], B2=[B|B]
            # (A = Wdd^T @ Wdis^T, B = Wdg^T @ Wdrug^T); C2/D2 come packed.
            # scales are folded into the packed-table copies below.
            a_ps = pspool.tile([F, P], f32, tag="abps")
            nc.tensor.matmul(out=a_ps[:], lhsT=wdd_t, rhs=d2, start=True, stop=True)
            a2 = cpool.tile([F, P], bf16, tag="a2")
            nc.vector.tensor_copy(out=a2[:], in_=a_ps[:])
            b_ps = pspool.tile([F, P], f32, tag="abps")
            nc.tensor.matmul(out=b_ps[:], lhsT=wdg_t, rhs=c2, start=True, stop=True)
            b2 = cpool.tile([F, P], bf16, tag="b2")
            nc.vector.tensor_copy(out=b2[:], in_=b_ps[:])

            # (k, e) -> (doubled lhsT AP, scale applied in the packed copy)
            m2 = {
                (0, 0): (c2, 0.5),   (0, 1): (a2[:], 0.125),
                (3, 0): (b2[:], 0.125), (3, 1): (d2, 0.5),
                (1, 0): (c2, 0.25),  (1, 1): (a2[:], 0.125),
                (2, 0): (b2[:], 0.125), (2, 1): (d2, 0.25),
            }

            # ---- build packed tables (order: mi, dr, ge)
            # tab[k][64h+p, r] = u32(bf16 T_k[r, p], bf16 T_k[r, p+64])
            tab = {k: cpool.tile([P, R], i32, tag=f"tab{k}", name=f"tab{k}")
                   for k in range(4)}
            tab_bf = {k: tab[k][:].bitcast(bf16) for k in range(4)}  # [P, 2R]

            feat_slots = {"mi": [0], "dr": [3], "ge": [1, 2]}
            # mi builds in 4 groups of 256 rows (shallower chain to the first
            # gather); dr/ge in 2 groups of 512
            ngrp = {"mi": 2, "dr": 2, "ge": 2}
            for name in order:
                ft = feat_ld[name]
                ng = ngrp[name]
                gr = R // ng
                # f32 -> bf16 (DVE/ACT alternating groups); host-transposed
                fb = ppool.tile([P, R], bf16, tag=f"fb_{name}",
                                name=f"fb_{name}")
                for grp in range(ng):
                    sl = slice(grp * gr, (grp + 1) * gr)
                    if grp % 2 == 0:
                        nc.vector.tensor_copy(out=fb[:, sl], in_=ft[:, sl])
                    else:
                        nc.scalar.activation(out=fb[:, sl], in_=ft[:, sl],
                                             func=mybir.ActivationFunctionType.Copy)
                    rhs = fb[:, sl]
                    for k in feat_slots[name]:
                        for e in range(2):
                            lhsT, scale = m2[(k, e)]
                            mm = mmpool.tile([P, R // 2], f32, tag="mmps")
                            nc.tensor.matmul(
                                out=mm[:, :gr], lhsT=lhsT, rhs=rhs,
                                start=True, stop=True,
                            )
                            st = grp * gr * 2 + e
                            dst = tab_bf[k][:, st:st + 2 * gr - 1:2]
                            if (k + e) % 2 == 0:
                                nc.vector.tensor_scalar_mul(dst, mm[:, :gr], scale)
                            else:
                                nc.scalar.activation(
                                    out=dst, in_=mm[:, :gr],
                                    func=mybir.ActivationFunctionType.Copy,
                                    scale=scale,
                                )

            # ---- main loop: ap_gather + add + store
            # chain the gathers so the Pool engine runs them in chunk order
            # (the list scheduler otherwise interleaves chunks, which
            # head-of-line blocks the in-order DVE queue on the adds)
            prev_gather = None
            prev_add = None
            last = len(CHUNKS) - 1
            for ch, ct in enumerate(CHUNKS):
                n = ct // 2
                o = OFFS[ch]
                g = {}
                # last chunk: gather the (late) gene slots first so only one
                # short add remains after the final gather
                korder = (1, 2, 0, 3) if ch == last else GORDER
                for k in korder:
                    gt = gpool.tile([P, n], i32, tag=f"g{k}_{n}",
                                    name=f"g{k}_{ch}")
                    gi = nc.gpsimd.ap_gather(
                        gt[:], tab[k][:], idx_t[:, k, o // 16:(o + n) // 16],
                        channels=P, num_elems=R, d=1, num_idxs=n,
                    )
                    if prev_gather is not None:
                        add_dep_helper(gi.ins, prev_gather.ins,
                                       reason="pool gather order")
                    prev_gather = gi
                    g[k] = gt
                def chained_add(o, x, y):
                    nonlocal prev_add
                    ai = nc.vector.tensor_add(o, x, y)
                    if prev_add is not None:
                        add_dep_helper(ai.ins, prev_add.ins, reason="dve add order")
                    prev_add = ai
                    return ai
                b0 = g[0][:].bitcast(bf16)
                b1 = g[1][:].bitcast(bf16)
                b2 = g[2][:].bitcast(bf16)
                b3 = g[3][:].bitcast(bf16)
                if ch < last:
                    # pair by arrival order (0,3 first, then 1,2)
                    chained_add(b0, b0, b3)
                    chained_add(b1, b1, b2)
                    chained_add(b0, b0, b1)
                    nc.sync.dma_start(out[:, o:o + n], g[0][:])
                else:
                    # arrival order is 1,2,0,3: g1+g2 then +g0 run during the
                    # g0/g3 gathers; only (+g3) trails, split 3/4 + 1/4 so
                    # most of the store pipeline overlaps the last quarter-add
                    hn = 3 * n // 4
                    chained_add(b1, b1, b2)
                    chained_add(b0, b0, b1)
                    chained_add(b0[:, :2 * hn], b0[:, :2 * hn],
                                         b3[:, :2 * hn])
                    nc.sync.dma_start(out[:, o:o + hn], g[0][:, :hn])
                    nc.vector.tensor_add(b0[:, 2 * hn:], b0[:, 2 * hn:],
                                         b3[:, 2 * hn:])
                    nc.sync.dma_start(out[:, o + hn:o + n], g[0][:, hn:])

    nc.compile()
    return nc


def _prep_inputs(feat_miRNA, feat_gene, feat_drug, W_drug_disease, W_disease_drug,
                 W_drug, W_dis, mp_ins):
    """Marshal full inputs into per-core in_maps (no arithmetic on values)."""
    def pad_rows_t(a):
        """First R rows, zero-padded, host-transposed to [F, R]."""
        a = np.asarray(a, dtype=np.float32)
        out = np.zeros((R, a.shape[1]), dtype=np.float32)
        out[: min(R, a.shape[0])] = a[:R]
        return np.ascontiguousarray(out.T)

    f_mi = pad_rows_t(feat_miRNA)
    f_ge = pad_rows_t(feat_gene)
    f_dr = pad_rows_t(feat_drug)
    wdd = np.asarray(W_drug_disease, np.float32)
    wdg = np.asarray(W_disease_drug, np.float32)
    wdrug = np.asarray(W_drug, np.float32)
    wdis = np.asarray(W_dis, np.float32)
    # packed weights: [Wdd | Wdg | C C | D D], C = Wdrug^T, D = Wdis^T
    w_all = np.empty((P, 4 * F), dtype=np.float32)
    w_all[:, 0:F] = wdd
    w_all[:, F:2 * F] = wdg
    w_all[:, 2 * F:2 * F + HH] = wdrug.T
    w_all[:, 2 * F + HH:3 * F] = wdrug.T
    w_all[:, 3 * F:3 * F + HH] = wdis.T
    w_all[:, 3 * F + HH:4 * F] = wdis.T

    mp = np.asarray(mp_ins)
    assert mp.shape == (B_PAIRS, BAG, 4), mp.shape
    mp_flat = mp.reshape(B_PAIRS * BAG, 4).astype(np.int16)

    in_maps = []
    for core in range(N_CORES):
        mp_core = mp_flat[core * TOK:(core + 1) * TOK]        # [TOK, 4]
        # token (ch, h, j) -> gather idx at partition 64h + 16g + j%16,
        # free slot (k, OFFS[ch]//16 + j//16), replicated for g in 0..3
        idx_arr = np.empty((P, 4, NI // 16), dtype=np.int16)
        pos = 0
        for ch, ct in enumerate(CHUNKS):
            n = ct // 2
            mpc = mp_core[pos:pos + ct].reshape(2, n // 16, 16, 4)  # [h, s, p, k]
            pos += ct
            o16 = OFFS[ch] // 16
            for h in range(2):
                # mpc[h] is [s, p16, k] -> want [p16, k, s]
                blk = mpc[h].transpose(1, 2, 0)
                for gi in range(4):
                    idx_arr[64 * h + 16 * gi:64 * h + 16 * (gi + 1), :,
                            o16:o16 + n // 16] = blk
        in_maps.append(
            {
                "feat_mi": f_mi,
                "feat_ge": f_ge,
                "feat_dr": f_dr,
                "w_all": w_all,
                "idx": idx_arr,
            }
        )
    return in_maps


def _decode_out(out_u32):
    """[128, NI] packed u32 -> [TOK, H] f32 (exact bf16 widening)."""
    res = np.empty((TOK, H), dtype=np.uint32)
    v = out_u32.astype(np.uint32).reshape(2, 64, NI)          # [h, p, j]
    lo = (v & np.uint32(0xFFFF)) << np.uint32(16)             # feat p
    hi = (v >> np.uint32(16)) << np.uint32(16)                # feat p + 64
    pos = 0
    for ch, ct in enumerate(CHUNKS):
        n = ct // 2
        o = OFFS[ch]
        # token (h, j) of this chunk = pos + h*n + j
        blk_lo = lo[:, :, o:o + n]                            # [h, p, j]
        blk_hi = hi[:, :, o:o + n]
        res[pos:pos + ct, :HH] = blk_lo.transpose(0, 2, 1).reshape(ct, HH)
        res[pos:pos + ct, HH:] = blk_hi.transpose(0, 2, 1).reshape(ct, HH)
        pos += ct
    return res.view(np.float32)


def _numpy_fallback(feat_miRNA, feat_gene, feat_drug, W_drug_disease,
                    W_disease_drug, W_drug, W_dis, mp_ins):
    mi = np.asarray(feat_miRNA, np.float32)[mp_ins[:, :, 0]]
    g1 = np.asarray(feat_gene, np.float32)[mp_ins[:, :, 1]]
    g2 = np.asarray(feat_gene, np.float32)[mp_ins[:, :, 2]]
    dr = np.asarray(feat_drug, np.float32)[mp_ins[:, :, 3]]
    wdd = np.asarray(W_drug_disease, np.float32)
    wdg = np.asarray(W_disease_drug, np.float32)
    wdrug = np.asarray(W_drug, np.float32)
    wdis = np.asarray(W_dis, np.float32)
    dis = ((((mi + g1) * 0.5) @ wdd.T + g2) * 0.5 + dr) * 0.5
    drug = ((((dr + g2) * 0.5) @ wdg.T + g1) * 0.5 + mi) * 0.5
    return np.concatenate([drug @ wdrug.T, dis @ wdis.T], axis=2)


def kernel(**inputs):
    mp = np.asarray(inputs["mp_ins"])
    if mp.max() >= R or mp.min() < 0:
        # outside the spec's index range; fall back to exact host compute
        return _numpy_fallback(**inputs)

    from concourse.bass_utils import run_bass_kernel_spmd

    if "nc" not in _CACHE:
        _CACHE["nc"] = _build_module()
    nc = _CACHE["nc"]

    in_maps = _prep_inputs(**inputs)
    res = run_bass_kernel_spmd(nc, in_maps, core_ids=list(range(N_CORES)))
    outs = [_decode_out(np.asarray(r["out"])) for r in res.results]
    return np.concatenate(outs, axis=0).reshape(B_PAIRS, BAG, H)


if __name__ == "__main__":
    import reference

    inputs = {k: np.asarray(v) for k, v in reference.setup_inputs().items()}
    expected = np.asarray(reference.reference(**inputs))
    actual = kernel(**inputs)
    err = np.abs(actual - expected).max() / (np.abs(expected).max() + 1e-9)
    print("max abs err (scaled):", err)
    rel = np.linalg.norm(actual - expected) / np.linalg.norm(expected)
    print("Relative error:", rel)


# revision 46
# speedup vs baseline: 1.0268x; 1.0043x over previous
"""MetaPathAggregator kernel for Trainium2 (8 NeuronCores, data-parallel).

Math: the reference module is linear in the four gathered feature rows:

    dis  = 0.125*(mi+g1)@Wdd^T + 0.25*g2 + 0.5*dr
    drug = 0.125*(dr+g2)@Wdg^T + 0.25*g1 + 0.5*mi
    out  = [drug @ Wdrug^T | dis @ Wdis^T]
         = mi@M_mi + g1@M_g1 + g2@M_g2 + dr@M_dr

with per-slot 128x128 matrices

    M_mi = [0.500*C | 0.125*A]      A = Wdd^T @ Wdis^T   (128x64)
    M_g1 = [0.250*C | 0.125*A]      B = Wdg^T @ Wdrug^T  (128x64)
    M_g2 = [0.125*B | 0.250*D]      C = Wdrug^T          (128x64)
    M_dr = [0.125*B | 0.500*D]      D = Wdis^T           (128x64)

Indices are < 1000 (spec fill_max), so only 1024 rows of each table are
live and the per-token work is out[t] = T_mi[i0]+T_g1[i1]+T_g2[i2]+T_dr[i3]
over four transformed 1024x128 tables.

Device schedule per core (16384 tokens): the tables are built IN SBUF in a
packed column-major bf16 layout and the per-token row lookups run on the
Pool engine via gpsimd.ap_gather (SBUF-local), bypassing the DMA engines
entirely (the bottleneck of a dma_gather design: every gathered row is a
512B DMA descriptor, ~93us/core of serialized DMA time).

Packed table layout (tab[k], int32 [128, 1024]):
    tab[k][64*h + p, r] = u32(lo=bf16(T_k[r, p]), hi=bf16(T_k[r, p+64]))
for p in 0..63; the h=0 and h=1 partition halves hold identical copies.
ap_gather applies an independent index list per 16-partition group, so one
ap_gather with num_idxs=N serves 2N tokens (half A on partitions 0-63,
half B on 64-127) at one charged u32 element per token (~1.4ns Pool each).

T_k^T is computed directly in packed form by parity matmuls with
lhsT = [M_k[:,64e:64e+64] | same] (both partition halves at once); the f32
PSUM result lands in the packed table via a stride-2 bf16 copy (DVE/ACT
alternating).  Tables build in order mi, dr, gene so the first chunks'
mi/dr gathers overlap the gene transform; chunk sizes shrink toward the
end so the final add+store tail is short.

The gathered chunk tiles are summed as bf16 views on DVE and stored packed;
the host decodes the bf16 pair bits to f32 (exact widening, no arithmetic).
"""

import numpy as np

P = 128          # partitions
F = 128          # input feature dim
H = 128          # output hidden dim
HH = 64          # half hidden
R = 1024         # padded table rows (indices < 1000)
N_CORES = 8
B_PAIRS = 1024
BAG = 128
TOK = B_PAIRS * BAG // N_CORES   # 16384 tokens per core
CHUNKS = [4096, 4096, 4096, 2048, 2048]         # tokens per chunk
assert sum(CHUNKS) == TOK
NS = [ct // 2 for ct in CHUNKS]                 # ap_gather num_idxs per chunk
OFFS = np.cumsum([0] + NS).tolist()             # idx free-dim offsets (/16 later)
NI = OFFS[-1]                                   # total idx per slot = TOK//2
GORDER = (0, 3, 1, 2)                           # gather slot order per chunk

_CACHE = {}


def _build_module():
    import concourse.bacc as bacc
    import concourse.mybir as mybir
    import concourse.tile as tile
    from concourse.masks import make_identity
    from concourse.tile_rust import add_dep_helper

    f32 = mybir.dt.float32
    bf16 = mybir.dt.bfloat16
    i32 = mybir.dt.int32
    i16 = mybir.dt.int16

    nc = bacc.Bacc("TRN2")

    # host-transposed feature tables [F, R] (partition = input feature)
    feat_in = {
        "mi": nc.dram_tensor("feat_mi", [F, R], f32, kind="ExternalInput"),
        "ge": nc.dram_tensor("feat_ge", [F, R], f32, kind="ExternalInput"),
        "dr": nc.dram_tensor("feat_dr", [F, R], f32, kind="ExternalInput"),
    }
    # host-packed: [Wdd | Wdg | C C | D D], C = Wdrug^T, D = Wdis^T
    w_all = nc.dram_tensor("w_all", [P, 4 * F], f32, kind="ExternalInput")
    idx_in = nc.dram_tensor("idx", [P, 4, NI // 16], i16, kind="ExternalInput")
    # packed u32 output, free-dim offset o..o+N per chunk
    out = nc.dram_tensor("out", [P, NI], i32, kind="ExternalOutput")

    with tile.TileContext(nc) as tc:
        with (
            tc.tile_pool(name="const", bufs=1) as cpool,
            tc.tile_pool(name="prep", bufs=2) as ppool,
            tc.tile_pool(name="psum", bufs=2, space="PSUM") as pspool,
            tc.tile_pool(name="psum_mm", bufs=4, space="PSUM") as mmpool,
            tc.tile_pool(name="gather", bufs=2) as gpool,
        ):
            # ---- weights first in the DMA queue: the weight -> doubled-matrix
            # chain is longer than the feat_mi -> transpose chain
            wl32 = ppool.tile([P, 4 * F], f32, tag="wload", bufs=2)
            nc.sync.dma_start(wl32[:], w_all[:, :])
            wall = cpool.tile([P, 4 * F], bf16, tag="wall")
            nc.vector.tensor_copy(out=wall[:, :2 * F], in_=wl32[:, :2 * F])
            nc.scalar.activation(out=wall[:, 2 * F:], in_=wl32[:, 2 * F:],
                                 func=mybir.ActivationFunctionType.Copy)
            wdd_t = wall[:, 0:F]
            wdg_t = wall[:, F:2 * F]
            c2 = wall[:, 2 * F:3 * F]          # [C|C]
            d2 = wall[:, 3 * F:4 * F]          # [D|D]

            feat_ld = {}
            order = ("mi", "dr", "ge")
            feat_ld["mi"] = ppool.tile([P, R], f32, tag="featld_mi",
                                       name="featld_mi", bufs=2)
            for q in range(2):
                nc.sync.dma_start(feat_ld["mi"][:, q * R // 2:(q + 1) * R // 2],
                                  feat_in["mi"][:, q * R // 2:(q + 1) * R // 2])

            idx_t = cpool.tile([P, 4, NI // 16], i16)
            nc.sync.dma_start(idx_t[:], idx_in[:, :, :])

            for name in ("dr", "ge"):
                ft = ppool.tile([P, R], f32, tag=f"featld_{name}",
                                name=f"featld_{name}", bufs=2)
                nc.sync.dma_start(ft[:], feat_in[name][:, :])
                feat_ld[name] = ft

            # ---- doubled matrices A2=[A|A], B2=[B|B]
            # (A = Wdd^T @ Wdis^T, B = Wdg^T @ Wdrug^T); C2/D2 come packed.
            # scales are folded into the packed-table copies below.
            a_ps = pspool.tile([F, P], f32, tag="abps")
            nc.tensor.matmul(out=a_ps[:], lhsT=wdd_t, rhs=d2, start=True, stop=True)
            a2 = cpool.tile([F, P], bf16, tag="a2")
            nc.vector.tensor_copy(out=a2[:], in_=a_ps[:])
            b_ps = pspool.tile([F, P], f32, tag="abps")
            nc.tensor.matmul(out=b_ps[:], lhsT=wdg_t, rhs=c2, start=True, stop=True)
            b2 = cpool.tile([F, P], bf16, tag="b2")
            nc.vector.tensor_copy(out=b2[:], in_=b_ps[:])

            # (k, e) -> (doubled lhsT AP, scale applied in the packed copy)
            m2 = {
                (0, 0): (c2, 0.5),   (0, 1): (a2[:], 0.125),
                (3, 0): (b2[:], 0.125), (3, 1): (d2, 0.5),
                (1, 0): (c2, 0.25),  (1, 1): (a2[:], 0.125),
                (2, 0): (b2[:], 0.125), (2, 1): (d2, 0.25),
            }

            # ---- build packed tables (order: mi, dr, ge)
            # tab[k][64h+p, r] = u32(bf16 T_k[r, p], bf16 T_k[r, p+64])
            tab = {k: cpool.tile([P, R], i32, tag=f"tab{k}", name=f"tab{k}")
                   for k in range(4)}
            tab_bf = {k: tab[k][:].bitcast(bf16) for k in range(4)}  # [P, 2R]

            feat_slots = {"mi": [0], "dr": [3], "ge": [1, 2]}
            # mi builds in 4 groups of 256 rows (shallower chain to the first
            # gather); dr/ge in 2 groups of 512
            ngrp = {"mi": 2, "dr": 2, "ge": 2}
            for name in order:
                ft = feat_ld[name]
                ng = ngrp[name]
                gr = R // ng
                # f32 -> bf16 (DVE/ACT alternating groups); host-transposed
                fb = ppool.tile([P, R], bf16, tag=f"fb_{name}",
                                name=f"fb_{name}")
                for grp in range(ng):
                    sl = slice(grp * gr, (grp + 1) * gr)
                    if grp % 2 == 0:
                        nc.vector.tensor_copy(out=fb[:, sl], in_=ft[:, sl])
                    else:
                        nc.scalar.activation(out=fb[:, sl], in_=ft[:, sl],
                                             func=mybir.ActivationFunctionType.Copy)
                    rhs = fb[:, sl]
                    for k in feat_slots[name]:
                        for e in range(2):
                            lhsT, scale = m2[(k, e)]
                            mm = mmpool.tile([P, R // 2], f32, tag="mmps")
                            nc.tensor.matmul(
                                out=mm[:, :gr], lhsT=lhsT, rhs=rhs,
                                start=True, stop=True,
                            )
                            st = grp * gr * 2 + e
                            dst = tab_bf[k][:, st:st + 2 * gr - 1:2]
                            if (k + e) % 2 == 0:
                                nc.vector.tensor_scalar_mul(dst, mm[:, :gr], scale)
                            else:
                                nc.scalar.activation(
                                    out=dst, in_=mm[:, :gr],
                                    func=mybir.ActivationFunctionType.Copy,
                                    scale=scale,
                                )

            # ---- main loop: ap_gather + add + store
            # chain the gathers so the Pool engine runs them in chunk order
            # (the list scheduler otherwise interleaves chunks, which
            # head-of-line blocks the in-order DVE queue on the adds)
            prev_gather = None
            prev_add = None
            last = len(CHUNKS) - 1
            for ch, ct in enumerate(CHUNKS):
                n = ct // 2
                o = OFFS[ch]
                g = {}
                # last chunk: gather the (late) gene slots first so only one
                # short add remains after the final gather
                korder = (1, 2, 0, 3) if ch == last else GORDER
                for k in korder:
                    gt = gpool.tile([P, n], i32, tag=f"g{k}_{n}",
                                    name=f"g{k}_{ch}")
                    gi = nc.gpsimd.ap_gather(
                        gt[:], tab[k][:], idx_t[:, k, o // 16:(o + n) // 16],
                        channels=P, num_elems=R, d=1, num_idxs=n,
                    )
                    if prev_gather is not None:
                        add_dep_helper(gi.ins, prev_gather.ins,
                                       reason="pool gather order")
                    prev_gather = gi
                    g[k] = gt
                def chained_add(o, x, y):
                    nonlocal prev_add
                    ai = nc.vector.tensor_add(o, x, y)
                    if prev_add is not None:
                        add_dep_helper(ai.ins, prev_add.ins, reason="dve add order")
                    prev_add = ai
                    return ai
                b0 = g[0][:].bitcast(bf16)
                b1 = g[1][:].bitcast(bf16)
                b2 = g[2][:].bitcast(bf16)
                b3 = g[3][:].bitcast(bf16)
                if ch < last:
                    # pair by arrival order (0,3 first, then 1,2)
                    chained_add(b0, b0, b3)
                    chained_add(b1, b1, b2)
                    chained_add(b0, b0, b1)
                    nc.sync.dma_start(out[:, o:o + n], g[0][:])
                else:
                    # arrival order is 1,2,0,3: g1+g2 then +g0 run during the
                    # g0/g3 gathers; only (+g3) trails, split in halves so
                    # the first store pipeline overlaps the second half-add
                    hn = n // 2
                    chained_add(b1, b1, b2)
                    chained_add(b0, b0, b1)
                    chained_add(b0[:, :2 * hn], b0[:, :2 * hn],
                                         b3[:, :2 * hn])
                    nc.sync.dma_start(out[:, o:o + hn], g[0][:, :hn])
                    nc.vector.tensor_add(b0[:, 2 * hn:], b0[:, 2 * hn:],
                                         b3[:, 2 * hn:])
                    nc.sync.dma_start(out[:, o + hn:o + n], g[0][:, hn:])

    nc.compile()
    return nc


def _prep_inputs(feat_miRNA, feat_gene, feat_drug, W_drug_disease, W_disease_drug,
                 W_drug, W_dis, mp_ins):
    """Marshal full inputs into per-core in_maps (no arithmetic on values)."""
    def pad_rows_t(a):
        """First R rows, zero-padded, host-transposed to [F, R]."""
        a = np.asarray(a, dtype=np.float32)
        out = np.zeros((R, a.shape[1]), dtype=np.float32)
        out[: min(R, a.shape[0])] = a[:R]
        return np.ascontiguousarray(out.T)

    f_mi = pad_rows_t(feat_miRNA)
    f_ge = pad_rows_t(feat_gene)
    f_dr = pad_rows_t(feat_drug)
    wdd = np.asarray(W_drug_disease, np.float32)
    wdg = np.asarray(W_disease_drug, np.float32)
    wdrug = np.asarray(W_drug, np.float32)
    wdis = np.asarray(W_dis, np.float32)
    # packed weights: [Wdd | Wdg | C C | D D], C = Wdrug^T, D = Wdis^T
    w_all = np.empty((P, 4 * F), dtype=np.float32)
    w_all[:, 0:F] = wdd
    w_all[:, F:2 * F] = wdg
    w_all[:, 2 * F:2 * F + HH] = wdrug.T
    w_all[:, 2 * F + HH:3 * F] = wdrug.T
    w_all[:, 3 * F:3 * F + HH] = wdis.T
    w_all[:, 3 * F + HH:4 * F] = wdis.T

    mp = np.asarray(mp_ins)
    assert mp.shape == (B_PAIRS, BAG, 4), mp.shape
    mp_flat = mp.reshape(B_PAIRS * BAG, 4).astype(np.int16)

    in_maps = []
    for core in range(N_CORES):
        mp_core = mp_flat[core * TOK:(core + 1) * TOK]        # [TOK, 4]
        # token (ch, h, j) -> gather idx at partition 64h + 16g + j%16,
        # free slot (k, OFFS[ch]//16 + j//16), replicated for g in 0..3
        idx_arr = np.empty((P, 4, NI // 16), dtype=np.int16)
        pos = 0
        for ch, ct in enumerate(CHUNKS):
            n = ct // 2
            mpc = mp_core[pos:pos + ct].reshape(2, n // 16, 16, 4)  # [h, s, p, k]
            pos += ct
            o16 = OFFS[ch] // 16
            for h in range(2):
                # mpc[h] is [s, p16, k] -> want [p16, k, s]
                blk = mpc[h].transpose(1, 2, 0)
                for gi in range(4):
                    idx_arr[64 * h + 16 * gi:64 * h + 16 * (gi + 1), :,
                            o16:o16 + n // 16] = blk
        in_maps.append(
            {
                "feat_mi": f_mi,
                "feat_ge": f_ge,
                "feat_dr": f_dr,
                "w_all": w_all,
                "idx": idx_arr,
            }
        )
    return in_maps


def _decode_out(out_u32):
    """[128, NI] packed u32 -> [TOK, H] f32 (exact bf16 widening)."""
    res = np.empty((TOK, H), dtype=np.uint32)
    v = out_u32.astype(np.uint32).reshape(2, 64, NI)          # [h, p, j]
    lo = (v & np.uint32(0xFFFF)) << np.uint32(16)             # feat p
    hi = (v >> np.uint32(16)) << np.uint32(16)                # feat p + 64
    pos = 0
    for ch, ct in enumerate(CHUNKS):
        n = ct // 2
        o = OFFS[ch]
        # token (h, j) of this chunk = pos + h*n + j
        blk_lo = lo[:, :, o:o + n]                            # [h, p, j]
        blk_hi = hi[:, :, o:o + n]
        res[pos:pos + ct, :HH] = blk_lo.transpose(0, 2, 1).reshape(ct, HH)
        res[pos:pos + ct, HH:] = blk_hi.transpose(0, 2, 1).reshape(ct, HH)
        pos += ct
    return res.view(np.float32)


def _numpy_fallback(feat_miRNA, feat_gene, feat_drug, W_drug_disease,
                    W_disease_drug, W_drug, W_dis, mp_ins):
    mi = np.asarray(feat_miRNA, np.float32)[mp_ins[:, :, 0]]
    g1 = np.asarray(feat_gene, np.float32)[mp_ins[:, :, 1]]
    g2 = np.asarray(feat_gene, np.float32)[mp_ins[:, :, 2]]
    dr = np.asarray(feat_drug, np.float32)[mp_ins[:, :, 3]]
    wdd = np.asarray(W_drug_disease, np.float32)
    wdg = np.asarray(W_disease_drug, np.float32)
    wdrug = np.asarray(W_drug, np.float32)
    wdis = np.asarray(W_dis, np.float32)
    dis = ((((mi + g1) * 0.5) @ wdd.T + g2) * 0.5 + dr) * 0.5
    drug = ((((dr + g2) * 0.5) @ wdg.T + g1) * 0.5 + mi) * 0.5
    return np.concatenate([drug @ wdrug.T, dis @ wdis.T], axis=2)


def kernel(**inputs):
    mp = np.asarray(inputs["mp_ins"])
    if mp.max() >= R or mp.min() < 0:
        # outside the spec's index range; fall back to exact host compute
        return _numpy_fallback(**inputs)

    from concourse.bass_utils import run_bass_kernel_spmd

    if "nc" not in _CACHE:
        _CACHE["nc"] = _build_module()
    nc = _CACHE["nc"]

    in_maps = _prep_inputs(**inputs)
    res = run_bass_kernel_spmd(nc, in_maps, core_ids=list(range(N_CORES)))
    outs = [_decode_out(np.asarray(r["out"])) for r in res.results]
    return np.concatenate(outs, axis=0).reshape(B_PAIRS, BAG, H)


if __name__ == "__main__":
    import reference

    inputs = {k: np.asarray(v) for k, v in reference.setup_inputs().items()}
    expected = np.asarray(reference.reference(**inputs))
    actual = kernel(**inputs)
    err = np.abs(actual - expected).max() / (np.abs(expected).max() + 1e-9)
    print("max abs err (scaled):", err)
    rel = np.linalg.norm(actual - expected) / np.linalg.norm(expected)
    print("Relative error:", rel)
